# revision 24
# baseline (speedup 1.0000x reference)
"""Trainium2 Bass kernel for ErosionP4 (P4 group-equivariant grayscale erosion).

Reference computation (shapes hardcoded):
  x: [B=4, G=4, H=96, W=96, C=4] fp32, kernel: [5, 5, 3, C=4, F=8] fp32
  out[b,g,h,w,f] = sum_c min_{k,dy,dx} ( ygp[b,g,k,h+dy,w+dx,c] - krev[g,dy,dx,k,c,f] )
  where ygp[b,g,k] = x[b, (g+k-1) mod 4] spatially padded with +inf and
  krev = the 4 planar rotations of the depth-rotated SE, spatially reversed.

Sharding: core -> (g = core//2, f-half = core%2).  Each core computes all 4
batches for one group-rotation g and 4 of the 8 filters.  All four batches
share the SE values for the core's g.

Packing "cp128": the (c, h) axes are flattened into a 384-row stream split
into 3 chunks of 128 partitions, so every DVE instruction runs with all 128
lanes busy.  The per-(tap,c,f) SE value varies across partitions within a
chunk, which is exactly what the per-partition scalar operand of
scalar_tensor_tensor supports.  The channel sum then happens on the host
(the c pieces are partition-misaligned on device).

Per (tap, f, chunk) one fused DVE instruction does the whole erosion update:
  scalar_tensor_tensor: acc = min(window - kk, acc).
"""

import os
from contextlib import ExitStack

import numpy as np

import concourse.bass as bass
import concourse.mybir as mybir
import concourse.tile as tile
from concourse.bass_utils import run_bass_kernel_spmd

B, G, H, W, C = 4, 4, 96, 96, 4
KH, KW, F = 5, 5, 8
PAD = 2
HP, WP = H + PAD * 2, W + PAD * 2  # 100, 100
NTAP = 3 * KH * KW  # 75
N_CORES = 8
NP = 4  # batches per core
NF = F // 2  # filters per core
NCHUNK = 3  # ceil(C*H / 128)

# Configuration (module-level so experiments can flip them; defaults = best).
CFG_DTYPE = os.environ.get("KCFG_DTYPE", "fp16")  # fp32 | fp16 | bf16
CFG_PACK = os.environ.get("KCFG_PACK", "cp128")  # h96 | cp128
CFG_GPSIMD = int(os.environ.get("KCFG_GPSIMD", "0"))  # of NF*NCHUNK (cp128) or C*NF (h96) columns on gpsimd

_DT = {
    "fp32": (mybir.dt.float32, np.float32, 1e30),
    "fp16": (mybir.dt.float16, np.float16, 30000.0),
    "bf16": (mybir.dt.bfloat16, None, 1e30),
}

_prog_cache = {}
LAST_RESULTS = None


def _np_dtype(name):
    if name == "bf16":
        import ml_dtypes

        return np.dtype(ml_dtypes.bfloat16)
    return np.dtype(_DT[name][1])


def _chunk_ranges(m):
    """(c, h0, h1, p0, p1) pieces of stream rows [128m, 128(m+1))."""
    out = []
    r = 128 * m
    while r < 128 * (m + 1):
        c, h = r // H, r % H
        h1 = min(H, h + 128 * (m + 1) - r)
        out.append((c, h, h1, r - 128 * m, r - 128 * m + (h1 - h)))
        r += h1 - h
    return out


def _build_program(dtype_name, pack, gpsimd_n):
    dt, _, _ = _DT[dtype_name]
    two_byte = dtype_name in ("fp16", "bf16")
    # The kernel-tail Drain must wait on every sem lane used; with 8 SWDGE
    # lanes + 3 engines it exceeds the CTRL struct's sync-wait capacity.
    # Cap the SWDGE completion-sem lanes for this build.
    import concourse.tile_sem_assignment as _tsa

    _orig_swdge = _tsa.NUM_SWDGE_GLOBAL_SEMS
    _tsa.NUM_SWDGE_GLOBAL_SEMS = 4
    try:
        return _build_program_inner(dtype_name, pack, gpsimd_n, dt, two_byte)
    finally:
        _tsa.NUM_SWDGE_GLOBAL_SEMS = _orig_swdge


class _SplitDrainTC(tile.TileContext):
    """TileContext whose kernel-tail drain is split into one drain per sem
    lane: the stock single Drain carries a wait for every lane used, which
    overflows the CTRL struct's sync-wait encoding on this compiler."""

    def _drain_and_barrier(self, tick_clock, wait_clock):
        from concourse.tile_sem_assignment import N_PROCS
        from concourse.vector_clock import ScopedClock, VectorClock

        gc = tick_clock.global_clock
        ticks = [gc[p] for p in range(N_PROCS)]
        for p in range(N_PROCS):
            if ticks[p] <= 0:
                continue
            sub = [ticks[q] if q == p else 0 for q in range(N_PROCS)]
            d = self.nc.sync.drain()
            wait_clock.add_sem_waits(d.ins, ScopedClock({None: VectorClock(sub)}))

        self.nc.all_engine_barrier()
        assert self.sems is not None
        popped = self.nc._tile_sem_poison_stack.pop()
        assert popped is self._sem_poison
        self.nc.clear_and_free_semaphores(list(self.sems.allocated().values()))
        self.nc.all_engine_barrier()


def _build_program_inner(dtype_name, pack, gpsimd_n, dt, two_byte):
    nc = bass.Bass()
    # Input planes: [k, c, h_pad, pair, w_pad]; for 2-byte dtypes a second
    # copy shifted by one w element keeps odd-dx windows 4B-aligned (DVE
    # 2x packed mode needs aligned step-1 operands).
    xin = nc.declare_dram_parameter("xin", [3, C, HP, NP, WP], dt, isOutput=False)

    if pack == "cp128":
        ncols = NF * NCHUNK  # engine-split granularity per tap
        nkk = NTAP * NF * NCHUNK
        kkin = nc.declare_dram_parameter("kk", [128, nkk], mybir.dt.float32, isOutput=False)
        yout = nc.declare_dram_parameter("yout", [NF, 128, NCHUNK, NP, W], dt, isOutput=True)
    else:
        ncols = C * NF
        nkk = NTAP * ncols
        kkin = nc.declare_dram_parameter("kk", [H, nkk], mybir.dt.float32, isOutput=False)
        yout = nc.declare_dram_parameter("yout", [H, NP, W, NF], mybir.dt.float32, isOutput=True)

    with _SplitDrainTC(nc) as tc, ExitStack() as ctx:
        pool = ctx.enter_context(tc.tile_pool(name="main", bufs=1))

        # Compute-instruction ISA slots can encode only ONE sync wait, so
        # "touch" every DMA'd region with a trivial op on each consuming
        # engine right after its DMA (one wait each); later compute
        # instructions then inherit the dependency through engine program
        # order and carry no waits of their own.
        # Distinct destination slots per touch: a shared destination would be a
        # same-engine WAW hazard, which costs this instruction's single wait slot.
        touch_v = pool.tile([1, 512], mybir.dt.float32, name="touch_v", tag="touch_v")
        touch_s = pool.tile([1, 512], mybir.dt.float32, name="touch_s", tag="touch_s")
        touch_g = pool.tile([1, 512], mybir.dt.float32, name="touch_g", tag="touch_g")
        tctr = [0, 0, 0]

        def _touch(t, p0=0, scalar_too=False):
            src = t[p0 : p0 + 1, 0:1]
            i = tctr[0] = tctr[0] + 1
            nc.vector.tensor_scalar_add(touch_v[0:1, i : i + 1], src, 0.0)
            if scalar_too and two_byte:
                i = tctr[1] = tctr[1] + 1
                nc.scalar.copy(touch_s[0:1, i : i + 1], src)
            i = tctr[2] = tctr[2] + 1
            nc.gpsimd.tensor_scalar_add(touch_g[0:1, i : i + 1], src, 0.0)

        NPART = 128 if pack == "cp128" else H

        # One HWDGE dma_start fans out over several HW queues, so a consumer
        # would need more sync waits than compute-instruction ISA slots can
        # encode; the software DGE (gpsimd engine) uses a single queue.
        dma = nc.gpsimd.dma_start

        # Compute-engine SBUF reads must start at partition 0/32/64/96, so the
        # dy window shift cannot be a partition offset — keep one dy-shifted
        # copy per (k, dy, chunk) (cp128) / (k, dy, c) (h96), sliced from HBM.
        # The odd-dx alignment copy (in_b = in_a shifted one element left) is
        # built by the otherwise-idle Scalar engine instead of more DMAs.
        in_a = {}
        in_b = {}

        def _load(key, src_k, src_c_or_ranges, dy):
            if pack == "cp128":
                t = pool.tile([128, NP, WP], dt, name=f"ina_{key}", tag=f"ina_{key}")
                for (c, h0, h1, p0, p1) in src_c_or_ranges:
                    dma(t[p0:p1], xin[src_k, c, h0 + dy : h1 + dy])
                    _touch(t[:, 0], p0, scalar_too=True)
            else:
                t = pool.tile([H, NP, WP], dt, name=f"ina_{key}", tag=f"ina_{key}")
                dma(t[:], xin[src_k, src_c_or_ranges, dy : dy + H])
                _touch(t[:, 0], 0, scalar_too=True)
            in_a[key] = t
            if two_byte:
                tb = pool.tile(list(t.shape), dt, name=f"inb_{key}", tag=f"inb_{key}")
                nc.scalar.copy(tb[:, :, 0 : WP - 1], t[:, :, 1:WP])
                _touch(tb[:, 0])
                in_b[key] = tb

        for k in range(3):
            for dy in range(KH):
                if pack == "cp128":
                    for m in range(NCHUNK):
                        _load((k, dy, m), k, _chunk_ranges(m), dy)
                else:
                    for c in range(C):
                        _load((k, dy, c), k, c, dy)

        kkt = pool.tile([NPART, nkk], mybir.dt.float32, name="kkt", tag="kkt")
        dma(kkt[:], kkin[:])
        _touch(kkt)

        accs = {}
        if pack == "cp128":
            # One tile per filter with the chunk index as a free dim, so the
            # output needs only NF=4 DMAs (<=8 HWDGE queues, no FIFO reuse wait).
            accf = {}
            for f in range(NF):
                accf[f] = pool.tile([128, NCHUNK, NP, W], dt, name=f"acc_{f}", tag=f"acc_{f}")
                for m in range(NCHUNK):
                    accs[f, m] = accf[f][:, m]
        else:
            for c in range(C):
                for f in range(NF):
                    accs[c, f] = pool.tile([H, NP, W], dt, name=f"acc_{c}_{f}", tag=f"acc_{c}_{f}")

        taps = [(k, dy, dx) for k in range(3) for dy in range(KH) for dx in range(KW)]

        def emit(ti, win_sel, acc, col):
            kk_ap = kkt[:, ti * ncols + col : ti * ncols + col + 1]
            eng = nc.gpsimd if col >= ncols - gpsimd_n else nc.vector
            if ti == 0:
                eng.tensor_scalar(acc[:], win_sel, kk_ap, None, mybir.AluOpType.subtract)
            else:
                eng.scalar_tensor_tensor(
                    acc[:], win_sel, kk_ap, acc[:],
                    mybir.AluOpType.subtract, mybir.AluOpType.min,
                )

        for ti, (k, dy, dx) in enumerate(taps):
            use_b = two_byte and (dx % 2 == 1)
            dxa = dx - 1 if use_b else dx
            if pack == "cp128":
                for f in range(NF):
                    for m in range(NCHUNK):
                        src = in_b[k, dy, m] if use_b else in_a[k, dy, m]
                        emit(ti, src[:, :, dxa : dxa + W], accs[f, m], f * NCHUNK + m)
            else:
                for c in range(C):
                    src = in_b[k, dy, c] if use_b else in_a[k, dy, c]
                    win = src[:, :, dxa : dxa + W]
                    for f in range(NF):
                        emit(ti, win, accs[c, f], c * NF + f)

        if pack == "cp128":
            # Channel sum happens on the host; just store the 12 acc tiles.
            for f in range(NF):
                # A Pool-engine touch absorbs the DVE dependency (1 wait), so
                # the SWDGE out-DMA dispatched next on the same sequencer needs
                # only its queue-FIFO wait.
                i = tctr[2] = tctr[2] + 1
                nc.gpsimd.tensor_scalar_add(touch_g[0:1, i : i + 1], accf[f][0:1, 0, 0, 0:1], 0.0)
                dma(yout[f], accf[f][:])

        else:
            out_t = pool.tile([H, NP, W, NF], mybir.dt.float32, name="out_t", tag="out_t")
            for f in range(NF):
                s1 = pool.tile([H, NP, W], mybir.dt.float32, name=f"s1_{f}", tag="s1", bufs=2)
                s2 = pool.tile([H, NP, W], mybir.dt.float32, name=f"s2_{f}", tag="s2", bufs=2)
                nc.vector.tensor_add(s1[:], accs[0, f][:], accs[1, f][:])
                nc.vector.tensor_add(s2[:], accs[2, f][:], accs[3, f][:])
                nc.vector.tensor_add(out_t[:, :, :, f], s1[:], s2[:])
            nc.sync.dma_start(yout[:], out_t[:])

    return nc


def _get_program(dtype_name, pack, gpsimd_n):
    key = (dtype_name, pack, gpsimd_n)
    if key not in _prog_cache:
        _prog_cache[key] = _build_program(dtype_name, pack, gpsimd_n)
    return _prog_cache[key]


def _krev(kernel):
    """[g, dy, dx, k, c, f] rotated/reversed SE, pure re-indexing of `kernel`."""
    k_ero = np.stack(
        [
            np.rot90(kernel[:, :, 2], k=3, axes=(0, 1)),
            kernel[:, :, 1],
            np.rot90(kernel[:, :, 0], k=1, axes=(0, 1)),
        ],
        axis=2,
    )
    krot = np.stack([np.rot90(k_ero, k=j, axes=(0, 1)) for j in range(4)], axis=0)
    return krot[:, ::-1, ::-1]


def _core_units(core):
    g = core // 2
    fh = core % 2
    return g, list(range(B)), list(range(fh * NF, fh * NF + NF))


def _make_in_map(x, kr, pack, core, np_dt, big, two_byte):
    g, bs, fs = _core_units(core)
    planes = np.full((3, C, HP, NP, WP), big, np.float32)
    for pi, b in enumerate(bs):
        for k in range(3):
            src = x[b, (g + k - 1) % 4]  # [H, W, C]
            planes[k, :, PAD : PAD + H, pi, PAD : PAD + W] = src.transpose(2, 0, 1)
    sel = kr[g][:, :, :, :, fs]  # [dy, dx, k, c, NF]
    taps_kcf = np.ascontiguousarray(sel.transpose(2, 0, 1, 3, 4))  # [k,dy,dx,c,NF]
    if pack == "cp128":
        # kk[p, (tap, f, m)] = kr[g, tap, c(m,p), f]
        tap_cf = taps_kcf.reshape(NTAP, C, NF)
        kk = np.empty((128, NTAP * NF * NCHUNK), np.float32)
        for m in range(NCHUNK):
            for (c, h0, h1, p0, p1) in _chunk_ranges(m):
                for ti in range(NTAP):
                    for f in range(NF):
                        kk[p0:p1, (ti * NF + f) * NCHUNK + m] = tap_cf[ti, c, f]
    else:
        kkflat = taps_kcf.reshape(-1)
        kk = np.ascontiguousarray(np.broadcast_to(kkflat, (H, kkflat.size)))
    return {"xin": planes.astype(np_dt), "kk": np.ascontiguousarray(kk)}


def _assemble(results, pack):
    out = np.zeros((B, G, H, W, F), np.float32)
    for core in range(N_CORES):
        g, bs, fs = _core_units(core)
        y = np.asarray(results[core]["yout"]).astype(np.float32)
        if pack == "cp128":
            # y: [NF, NCHUNK, 128, NP, W]; sum the c pieces into out
            for fi, f in enumerate(fs):
                for m in range(NCHUNK):
                    for (c, h0, h1, p0, p1) in _chunk_ranges(m):
                        for pi, b in enumerate(bs):
                            out[b, g, h0:h1, :, f] += y[fi, p0:p1, m, pi, :]
        else:
            for pi, b in enumerate(bs):
                out[b, g, :, :, fs[0] : fs[0] + len(fs)] = y[:, pi]
    return out


def kernel(x, kernel):
    x = np.ascontiguousarray(np.asarray(x, dtype=np.float32))
    se = np.ascontiguousarray(np.asarray(kernel, dtype=np.float32))
    dtype_name, pack, gpsimd_n = CFG_DTYPE, CFG_PACK, CFG_GPSIMD
    np_dt = _np_dtype(dtype_name)
    big = _DT[dtype_name][2]
    two_byte = dtype_name in ("fp16", "bf16")

    kr = _krev(se)  # [g, dy, dx, k, c, f]
    in_maps = [
        _make_in_map(x, kr, pack, core, np_dt, big, two_byte) for core in range(N_CORES)
    ]

    nc = _get_program(dtype_name, pack, gpsimd_n)
    res = run_bass_kernel_spmd(nc, in_maps, list(range(N_CORES)), trace=False)
    global LAST_RESULTS
    LAST_RESULTS = res
    return _assemble(res.results, pack)


# revision 25
# speedup vs baseline: 187.4967x; 187.4967x over previous
"""Trainium2 Bass kernel for ErosionP4 (P4 group-equivariant grayscale erosion).

Reference computation (shapes hardcoded):
  x: [B=4, G=4, H=96, W=96, C=4] fp32, kernel: [5, 5, 3, C=4, F=8] fp32
  out[b,g,h,w,f] = sum_c min_{k,dy,dx} ( ygp[b,g,k,h+dy,w+dx,c] - krev[g,dy,dx,k,c,f] )
  where ygp[b,g,k] = x[b, (g+k-1) mod 4] spatially padded with +inf and
  krev = the 4 planar rotations of the depth-rotated SE, spatially reversed.

Sharding: core -> (g = core//2, f-half = core%2).  Each core computes all 4
batches for one group-rotation g and 4 of the 8 filters.  All four batches
share the SE values for the core's g.

Packing "cp128": the (c, h) axes are flattened into a 384-row stream split
into 3 chunks of 128 partitions, so every DVE instruction runs with all 128
lanes busy.  The per-(tap,c,f) SE value varies across partitions within a
chunk, which is exactly what the per-partition scalar operand of
scalar_tensor_tensor supports.  The channel sum then happens on the host
(the c pieces are partition-misaligned on device).

Per (tap, f, chunk) one fused DVE instruction does the whole erosion update:
  scalar_tensor_tensor: acc = min(window - kk, acc).
"""

import os
from contextlib import ExitStack

import numpy as np

import concourse.bass as bass
import concourse.mybir as mybir
import concourse.tile as tile
from concourse.bass_utils import run_bass_kernel_spmd

B, G, H, W, C = 4, 4, 96, 96, 4
KH, KW, F = 5, 5, 8
PAD = 2
HP, WP = H + PAD * 2, W + PAD * 2  # 100, 100
NTAP = 3 * KH * KW  # 75
N_CORES = 8
NP = 4  # batches per core
NF = F // 2  # filters per core
NCHUNK = 3  # ceil(C*H / 128)

# Configuration (module-level so experiments can flip them; defaults = best).
CFG_DTYPE = os.environ.get("KCFG_DTYPE", "fp16")  # fp32 | fp16 | bf16
CFG_PACK = os.environ.get("KCFG_PACK", "cp128")  # h96 | cp128
CFG_GPSIMD = int(os.environ.get("KCFG_GPSIMD", "0"))  # of NF*NCHUNK (cp128) or C*NF (h96) columns on gpsimd
CFG_REPEAT = int(os.environ.get("KCFG_REPEAT", "1"))  # repeat compute on-device (timing slope runs)

_DT = {
    "fp32": (mybir.dt.float32, np.float32, 1e30),
    "fp16": (mybir.dt.float16, np.float16, 30000.0),
    "bf16": (mybir.dt.bfloat16, None, 1e30),
}

_prog_cache = {}
LAST_RESULTS = None


def _np_dtype(name):
    if name == "bf16":
        import ml_dtypes

        return np.dtype(ml_dtypes.bfloat16)
    return np.dtype(_DT[name][1])


def _chunk_ranges(m):
    """(c, h0, h1, p0, p1) pieces of stream rows [128m, 128(m+1))."""
    out = []
    r = 128 * m
    while r < 128 * (m + 1):
        c, h = r // H, r % H
        h1 = min(H, h + 128 * (m + 1) - r)
        out.append((c, h, h1, r - 128 * m, r - 128 * m + (h1 - h)))
        r += h1 - h
    return out


def _build_program(dtype_name, pack, gpsimd_n, repeat=1):
    dt, _, _ = _DT[dtype_name]
    two_byte = dtype_name in ("fp16", "bf16")
    # The kernel-tail Drain must wait on every sem lane used; with 8 SWDGE
    # lanes + 3 engines it exceeds the CTRL struct's sync-wait capacity.
    # Cap the SWDGE completion-sem lanes for this build.
    import concourse.tile_sem_assignment as _tsa

    _orig_swdge = _tsa.NUM_SWDGE_GLOBAL_SEMS
    _tsa.NUM_SWDGE_GLOBAL_SEMS = 4
    try:
        return _build_program_inner(dtype_name, pack, gpsimd_n, dt, two_byte, repeat)
    finally:
        _tsa.NUM_SWDGE_GLOBAL_SEMS = _orig_swdge


class _SplitDrainTC(tile.TileContext):
    """TileContext whose kernel-tail drain is split into one drain per sem
    lane: the stock single Drain carries a wait for every lane used, which
    overflows the CTRL struct's sync-wait encoding on this compiler."""

    def _drain_and_barrier(self, tick_clock, wait_clock):
        from concourse.tile_sem_assignment import N_PROCS
        from concourse.vector_clock import ScopedClock, VectorClock

        gc = tick_clock.global_clock
        ticks = [gc[p] for p in range(N_PROCS)]
        for p in range(N_PROCS):
            if ticks[p] <= 0:
                continue
            sub = [ticks[q] if q == p else 0 for q in range(N_PROCS)]
            d = self.nc.sync.drain()
            wait_clock.add_sem_waits(d.ins, ScopedClock({None: VectorClock(sub)}))

        self.nc.all_engine_barrier()
        assert self.sems is not None
        popped = self.nc._tile_sem_poison_stack.pop()
        assert popped is self._sem_poison
        self.nc.clear_and_free_semaphores(list(self.sems.allocated().values()))
        self.nc.all_engine_barrier()


def _build_program_inner(dtype_name, pack, gpsimd_n, dt, two_byte, repeat=1):
    nc = bass.Bass()
    # Input planes: [k, c, h_pad, pair, w_pad]; for 2-byte dtypes a second
    # copy shifted by one w element keeps odd-dx windows 4B-aligned (DVE
    # 2x packed mode needs aligned step-1 operands).
    xin = nc.declare_dram_parameter("xin", [3, C, HP, NP, WP], dt, isOutput=False)

    if pack == "cp128":
        ncols = NF * NCHUNK  # engine-split granularity per tap
        nkk = NTAP * NF * NCHUNK
        kkin = nc.declare_dram_parameter("kk", [128, nkk], mybir.dt.float32, isOutput=False)
        yout = nc.declare_dram_parameter("yout", [NF, 128, NCHUNK, NP, W], dt, isOutput=True)
    else:
        ncols = C * NF
        nkk = NTAP * ncols
        kkin = nc.declare_dram_parameter("kk", [H, nkk], mybir.dt.float32, isOutput=False)
        yout = nc.declare_dram_parameter("yout", [H, NP, W, NF], mybir.dt.float32, isOutput=True)

    with _SplitDrainTC(nc) as tc, ExitStack() as ctx:
        pool = ctx.enter_context(tc.tile_pool(name="main", bufs=1))

        # Compute-instruction ISA slots can encode only ONE sync wait, so
        # "touch" every DMA'd region with a trivial op on each consuming
        # engine right after its DMA (one wait each); later compute
        # instructions then inherit the dependency through engine program
        # order and carry no waits of their own.
        # Distinct destination slots per touch: a shared destination would be a
        # same-engine WAW hazard, which costs this instruction's single wait slot.
        touch_v = pool.tile([1, 512], mybir.dt.float32, name="touch_v", tag="touch_v")
        touch_s = pool.tile([1, 512], mybir.dt.float32, name="touch_s", tag="touch_s")
        touch_g = pool.tile([1, 512], mybir.dt.float32, name="touch_g", tag="touch_g")
        tctr = [0, 0, 0]

        def _touch(t, p0=0, scalar_too=False):
            src = t[p0 : p0 + 1, 0:1]
            i = tctr[0] = tctr[0] + 1
            nc.vector.tensor_scalar_add(touch_v[0:1, i : i + 1], src, 0.0)
            if scalar_too and two_byte:
                i = tctr[1] = tctr[1] + 1
                nc.scalar.copy(touch_s[0:1, i : i + 1], src)
            i = tctr[2] = tctr[2] + 1
            nc.gpsimd.tensor_scalar_add(touch_g[0:1, i : i + 1], src, 0.0)

        NPART = 128 if pack == "cp128" else H

        # One HWDGE dma_start fans out over several HW queues, so a consumer
        # would need more sync waits than compute-instruction ISA slots can
        # encode; the software DGE (gpsimd engine) uses a single queue.
        dma = nc.gpsimd.dma_start

        # Compute-engine SBUF reads must start at partition 0/32/64/96, so the
        # dy window shift cannot be a partition offset — keep one dy-shifted
        # copy per (k, dy, chunk) (cp128) / (k, dy, c) (h96), sliced from HBM.
        # The odd-dx alignment copy (in_b = in_a shifted one element left) is
        # built by the otherwise-idle Scalar engine instead of more DMAs.
        in_a = {}
        in_b = {}

        def _load(key, src_k, src_c_or_ranges, dy):
            if pack == "cp128":
                t = pool.tile([128, NP, WP], dt, name=f"ina_{key}", tag=f"ina_{key}")
                for (c, h0, h1, p0, p1) in src_c_or_ranges:
                    dma(t[p0:p1], xin[src_k, c, h0 + dy : h1 + dy])
                    _touch(t[:, 0], p0, scalar_too=True)
            else:
                t = pool.tile([H, NP, WP], dt, name=f"ina_{key}", tag=f"ina_{key}")
                dma(t[:], xin[src_k, src_c_or_ranges, dy : dy + H])
                _touch(t[:, 0], 0, scalar_too=True)
            in_a[key] = t
            if two_byte:
                tb = pool.tile(list(t.shape), dt, name=f"inb_{key}", tag=f"inb_{key}")
                nc.scalar.copy(tb[:, :, 0 : WP - 1], t[:, :, 1:WP])
                _touch(tb[:, 0])
                in_b[key] = tb

        for k in range(3):
            for dy in range(KH):
                if pack == "cp128":
                    for m in range(NCHUNK):
                        _load((k, dy, m), k, _chunk_ranges(m), dy)
                else:
                    for c in range(C):
                        _load((k, dy, c), k, c, dy)

        kkt = pool.tile([NPART, nkk], mybir.dt.float32, name="kkt", tag="kkt")
        dma(kkt[:], kkin[:])
        _touch(kkt)

        accs = {}
        if pack == "cp128":
            # One tile per filter with the chunk index as a free dim, so the
            # output needs only NF=4 DMAs (<=8 HWDGE queues, no FIFO reuse wait).
            accf = {}
            for f in range(NF):
                accf[f] = pool.tile([128, NCHUNK, NP, W], dt, name=f"acc_{f}", tag=f"acc_{f}")
                for m in range(NCHUNK):
                    accs[f, m] = accf[f][:, m]
        else:
            for c in range(C):
                for f in range(NF):
                    accs[c, f] = pool.tile([H, NP, W], dt, name=f"acc_{c}_{f}", tag=f"acc_{c}_{f}")

        taps = [(k, dy, dx) for k in range(3) for dy in range(KH) for dx in range(KW)]

        def emit(ti, win_sel, acc, col):
            kk_ap = kkt[:, ti * ncols + col : ti * ncols + col + 1]
            eng = nc.gpsimd if col >= ncols - gpsimd_n else nc.vector
            if ti == 0:
                eng.tensor_scalar(acc[:], win_sel, kk_ap, None, mybir.AluOpType.subtract)
            else:
                eng.scalar_tensor_tensor(
                    acc[:], win_sel, kk_ap, acc[:],
                    mybir.AluOpType.subtract, mybir.AluOpType.min,
                )

        for _rep in range(repeat):
          for ti, (k, dy, dx) in enumerate(taps):
            use_b = two_byte and (dx % 2 == 1)
            dxa = dx - 1 if use_b else dx
            if pack == "cp128":
                for f in range(NF):
                    for m in range(NCHUNK):
                        src = in_b[k, dy, m] if use_b else in_a[k, dy, m]
                        emit(ti, src[:, :, dxa : dxa + W], accs[f, m], f * NCHUNK + m)
            else:
                for c in range(C):
                    src = in_b[k, dy, c] if use_b else in_a[k, dy, c]
                    win = src[:, :, dxa : dxa + W]
                    for f in range(NF):
                        emit(ti, win, accs[c, f], c * NF + f)

        if pack == "cp128":
            # Channel sum happens on the host; just store the 12 acc tiles.
            for f in range(NF):
                # A Pool-engine touch absorbs the DVE dependency (1 wait), so
                # the SWDGE out-DMA dispatched next on the same sequencer needs
                # only its queue-FIFO wait.
                i = tctr[2] = tctr[2] + 1
                nc.gpsimd.tensor_scalar_add(touch_g[0:1, i : i + 1], accf[f][0:1, 0, 0, 0:1], 0.0)
                dma(yout[f], accf[f][:])

        else:
            out_t = pool.tile([H, NP, W, NF], mybir.dt.float32, name="out_t", tag="out_t")
            for f in range(NF):
                s1 = pool.tile([H, NP, W], mybir.dt.float32, name=f"s1_{f}", tag="s1", bufs=2)
                s2 = pool.tile([H, NP, W], mybir.dt.float32, name=f"s2_{f}", tag="s2", bufs=2)
                nc.vector.tensor_add(s1[:], accs[0, f][:], accs[1, f][:])
                nc.vector.tensor_add(s2[:], accs[2, f][:], accs[3, f][:])
                nc.vector.tensor_add(out_t[:, :, :, f], s1[:], s2[:])
            nc.sync.dma_start(yout[:], out_t[:])

    return nc


def _get_program(dtype_name, pack, gpsimd_n, repeat=1):
    key = (dtype_name, pack, gpsimd_n, repeat)
    if key not in _prog_cache:
        _prog_cache[key] = _build_program(dtype_name, pack, gpsimd_n, repeat)
    return _prog_cache[key]


def _krev(kernel):
    """[g, dy, dx, k, c, f] rotated/reversed SE, pure re-indexing of `kernel`."""
    k_ero = np.stack(
        [
            np.rot90(kernel[:, :, 2], k=3, axes=(0, 1)),
            kernel[:, :, 1],
            np.rot90(kernel[:, :, 0], k=1, axes=(0, 1)),
        ],
        axis=2,
    )
    krot = np.stack([np.rot90(k_ero, k=j, axes=(0, 1)) for j in range(4)], axis=0)
    return krot[:, ::-1, ::-1]


def _core_units(core):
    g = core // 2
    fh = core % 2
    return g, list(range(B)), list(range(fh * NF, fh * NF + NF))


def _make_in_map(x, kr, pack, core, np_dt, big, two_byte):
    g, bs, fs = _core_units(core)
    planes = np.full((3, C, HP, NP, WP), big, np.float32)
    for pi, b in enumerate(bs):
        for k in range(3):
            src = x[b, (g + k - 1) % 4]  # [H, W, C]
            planes[k, :, PAD : PAD + H, pi, PAD : PAD + W] = src.transpose(2, 0, 1)
    sel = kr[g][:, :, :, :, fs]  # [dy, dx, k, c, NF]
    taps_kcf = np.ascontiguousarray(sel.transpose(2, 0, 1, 3, 4))  # [k,dy,dx,c,NF]
    if pack == "cp128":
        # kk[p, (tap, f, m)] = kr[g, tap, c(m,p), f]
        tap_cf = taps_kcf.reshape(NTAP, C, NF)
        kk = np.empty((128, NTAP * NF * NCHUNK), np.float32)
        for m in range(NCHUNK):
            for (c, h0, h1, p0, p1) in _chunk_ranges(m):
                for ti in range(NTAP):
                    for f in range(NF):
                        kk[p0:p1, (ti * NF + f) * NCHUNK + m] = tap_cf[ti, c, f]
    else:
        kkflat = taps_kcf.reshape(-1)
        kk = np.ascontiguousarray(np.broadcast_to(kkflat, (H, kkflat.size)))
    return {"xin": planes.astype(np_dt), "kk": np.ascontiguousarray(kk)}


def _assemble(results, pack):
    out = np.zeros((B, G, H, W, F), np.float32)
    for core in range(N_CORES):
        g, bs, fs = _core_units(core)
        y = np.asarray(results[core]["yout"]).astype(np.float32)
        if pack == "cp128":
            # y: [NF, NCHUNK, 128, NP, W]; sum the c pieces into out
            for fi, f in enumerate(fs):
                for m in range(NCHUNK):
                    for (c, h0, h1, p0, p1) in _chunk_ranges(m):
                        for pi, b in enumerate(bs):
                            out[b, g, h0:h1, :, f] += y[fi, p0:p1, m, pi, :]
        else:
            for pi, b in enumerate(bs):
                out[b, g, :, :, fs[0] : fs[0] + len(fs)] = y[:, pi]
    return out


def kernel(x, kernel):
    x = np.ascontiguousarray(np.asarray(x, dtype=np.float32))
    se = np.ascontiguousarray(np.asarray(kernel, dtype=np.float32))
    dtype_name, pack, gpsimd_n = CFG_DTYPE, CFG_PACK, CFG_GPSIMD
    np_dt = _np_dtype(dtype_name)
    big = _DT[dtype_name][2]
    two_byte = dtype_name in ("fp16", "bf16")

    kr = _krev(se)  # [g, dy, dx, k, c, f]
    in_maps = [
        _make_in_map(x, kr, pack, core, np_dt, big, two_byte) for core in range(N_CORES)
    ]

    nc = _get_program(dtype_name, pack, gpsimd_n, CFG_REPEAT)
    res = run_bass_kernel_spmd(nc, in_maps, list(range(N_CORES)), trace=False)
    global LAST_RESULTS
    LAST_RESULTS = res
    return _assemble(res.results, pack)


# revision 30
# speedup vs baseline: 246.6267x; 1.3154x over previous
"""Trainium2 Bass kernel for ErosionP4 (P4 group-equivariant grayscale erosion).

Reference computation (shapes hardcoded):
  x: [B=4, G=4, H=96, W=96, C=4] fp32, kernel: [5, 5, 3, C=4, F=8] fp32
  out[b,g,h,w,f] = sum_c min_{k,dy,dx} ( ygp[b,g,k,h+dy,w+dx,c] - krev[g,dy,dx,k,c,f] )
  where ygp[b,g,k] = x[b, (g+k-1) mod 4] spatially padded with +inf and
  krev = the 4 planar rotations of the depth-rotated SE, spatially reversed.

Sharding: core -> (g = core//2, f-half = core%2).  Each core computes all 4
batches for one group-rotation g and 4 of the 8 filters.  All four batches
share the SE values for the core's g.

Packing "cp128": the (c, h) axes are flattened into a 384-row stream split
into 3 chunks of 128 partitions, so every DVE instruction runs with all 128
lanes busy.  The per-(tap,c,f) SE value varies across partitions within a
chunk, which is exactly what the per-partition scalar operand of
scalar_tensor_tensor supports.  The channel sum then happens on the host
(the c pieces are partition-misaligned on device).

Per (tap, f, chunk) one fused DVE instruction does the whole erosion update:
  scalar_tensor_tensor: acc = min(window - kk, acc).
"""

import os
from contextlib import ExitStack

import numpy as np

import concourse.bass as bass
import concourse.mybir as mybir
import concourse.tile as tile
from concourse.bass_utils import run_bass_kernel_spmd

B, G, H, W, C = 4, 4, 96, 96, 4
KH, KW, F = 5, 5, 8
PAD = 2
HP, WP = H + PAD * 2, W + PAD * 2  # 100, 100
NTAP = 3 * KH * KW  # 75
N_CORES = 8
NP = 4  # batches per core
NF = F // 2  # filters per core
NCHUNK = 3  # ceil(C*H / 128)

# Configuration (module-level so experiments can flip them; defaults = best).
CFG_DTYPE = os.environ.get("KCFG_DTYPE", "fp16")  # fp32 | fp16 | bf16
CFG_PACK = os.environ.get("KCFG_PACK", "cp128")  # h96 | cp128
CFG_GPSIMD = int(os.environ.get("KCFG_GPSIMD", "0"))  # of NF*NCHUNK (cp128) or C*NF (h96) columns on gpsimd
CFG_REPEAT = int(os.environ.get("KCFG_REPEAT", "1"))  # repeat compute on-device (timing slope runs)
CFG_SPLIT = int(os.environ.get("KCFG_SPLIT", "1"))  # 1: unfused ts+tt (2x/4x uops); 0: fused scalar_tensor_tensor

_DT = {
    "fp32": (mybir.dt.float32, np.float32, 1e30),
    "fp16": (mybir.dt.float16, np.float16, 30000.0),
    "bf16": (mybir.dt.bfloat16, None, 1e30),
}

_prog_cache = {}
LAST_RESULTS = None


def _np_dtype(name):
    if name == "bf16":
        import ml_dtypes

        return np.dtype(ml_dtypes.bfloat16)
    return np.dtype(_DT[name][1])


def _chunk_ranges(m):
    """(c, h0, h1, p0, p1) pieces of stream rows [128m, 128(m+1))."""
    out = []
    r = 128 * m
    while r < 128 * (m + 1):
        c, h = r // H, r % H
        h1 = min(H, h + 128 * (m + 1) - r)
        out.append((c, h, h1, r - 128 * m, r - 128 * m + (h1 - h)))
        r += h1 - h
    return out


def _build_program(dtype_name, pack, gpsimd_n, repeat=1):
    dt, _, _ = _DT[dtype_name]
    two_byte = dtype_name in ("fp16", "bf16")
    # The kernel-tail Drain must wait on every sem lane used; with 8 SWDGE
    # lanes + 3 engines it exceeds the CTRL struct's sync-wait capacity.
    # Cap the SWDGE completion-sem lanes for this build.
    import concourse.tile_sem_assignment as _tsa

    _orig_swdge = _tsa.NUM_SWDGE_GLOBAL_SEMS
    _tsa.NUM_SWDGE_GLOBAL_SEMS = 4
    try:
        return _build_program_inner(dtype_name, pack, gpsimd_n, dt, two_byte, repeat)
    finally:
        _tsa.NUM_SWDGE_GLOBAL_SEMS = _orig_swdge


class _SplitDrainTC(tile.TileContext):
    """TileContext whose kernel-tail drain is split into one drain per sem
    lane: the stock single Drain carries a wait for every lane used, which
    overflows the CTRL struct's sync-wait encoding on this compiler."""

    def _drain_and_barrier(self, tick_clock, wait_clock):
        from concourse.tile_sem_assignment import N_PROCS
        from concourse.vector_clock import ScopedClock, VectorClock

        gc = tick_clock.global_clock
        ticks = [gc[p] for p in range(N_PROCS)]
        for p in range(N_PROCS):
            if ticks[p] <= 0:
                continue
            sub = [ticks[q] if q == p else 0 for q in range(N_PROCS)]
            d = self.nc.sync.drain()
            wait_clock.add_sem_waits(d.ins, ScopedClock({None: VectorClock(sub)}))

        self.nc.all_engine_barrier()
        assert self.sems is not None
        popped = self.nc._tile_sem_poison_stack.pop()
        assert popped is self._sem_poison
        self.nc.clear_and_free_semaphores(list(self.sems.allocated().values()))
        self.nc.all_engine_barrier()


def _build_program_inner(dtype_name, pack, gpsimd_n, dt, two_byte, repeat=1):
    nc = bass.Bass()
    # Input planes: [k, c, h_pad, pair, w_pad]; for 2-byte dtypes a second
    # copy shifted by one w element keeps odd-dx windows 4B-aligned (DVE
    # 2x packed mode needs aligned step-1 operands).
    xin = nc.declare_dram_parameter("xin", [3, C, HP, NP, WP], dt, isOutput=False)

    if pack == "cp128":
        ncols = NF * NCHUNK  # engine-split granularity per tap
        nkk = NTAP * NF * NCHUNK
        kkin = nc.declare_dram_parameter("kk", [128, 2 * nkk], mybir.dt.float32, isOutput=False)
        yout = nc.declare_dram_parameter("yout", [NF, 128, NCHUNK, NP, W], dt, isOutput=True)
    else:
        ncols = C * NF
        nkk = NTAP * ncols
        kkin = nc.declare_dram_parameter("kk", [H, nkk], mybir.dt.float32, isOutput=False)
        yout = nc.declare_dram_parameter("yout", [H, NP, W, NF], mybir.dt.float32, isOutput=True)

    with _SplitDrainTC(nc) as tc, ExitStack() as ctx:
        pool = ctx.enter_context(tc.tile_pool(name="main", bufs=1))

        # Compute-instruction ISA slots can encode only ONE sync wait, so
        # "touch" every DMA'd region with a trivial op on each consuming
        # engine right after its DMA (one wait each); later compute
        # instructions then inherit the dependency through engine program
        # order and carry no waits of their own.
        # Distinct destination slots per touch: a shared destination would be a
        # same-engine WAW hazard, which costs this instruction's single wait slot.
        touch_v = pool.tile([1, 512], mybir.dt.float32, name="touch_v", tag="touch_v")
        touch_s = pool.tile([1, 512], mybir.dt.float32, name="touch_s", tag="touch_s")
        touch_g = pool.tile([1, 512], mybir.dt.float32, name="touch_g", tag="touch_g")
        tctr = [0, 0, 0]

        def _touch(t, p0=0, scalar_too=False):
            src = t[p0 : p0 + 1, 0:1]
            i = tctr[0] = tctr[0] + 1
            nc.vector.tensor_scalar_add(touch_v[0:1, i : i + 1], src, 0.0)
            if scalar_too and two_byte:
                i = tctr[1] = tctr[1] + 1
                nc.scalar.copy(touch_s[0:1, i : i + 1], src)
            i = tctr[2] = tctr[2] + 1
            nc.gpsimd.tensor_scalar_add(touch_g[0:1, i : i + 1], src, 0.0)

        NPART = 128 if pack == "cp128" else H

        # One HWDGE dma_start fans out over several HW queues, so a consumer
        # would need more sync waits than compute-instruction ISA slots can
        # encode; the software DGE (gpsimd engine) uses a single queue.
        dma = nc.gpsimd.dma_start

        # Compute-engine SBUF reads must start at partition 0/32/64/96, so the
        # dy window shift cannot be a partition offset — keep one dy-shifted
        # copy per (k, dy, chunk) (cp128) / (k, dy, c) (h96), sliced from HBM.
        # The odd-dx alignment copy (in_b = in_a shifted one element left) is
        # built by the otherwise-idle Scalar engine instead of more DMAs.
        in_a = {}
        in_b = {}

        def _load(key, src_k, src_c_or_ranges, dy):
            if pack == "cp128":
                t = pool.tile([128, NP, WP], dt, name=f"ina_{key}", tag=f"ina_{key}")
                for (c, h0, h1, p0, p1) in src_c_or_ranges:
                    dma(t[p0:p1], xin[src_k, c, h0 + dy : h1 + dy])
                    _touch(t[:, 0], p0, scalar_too=True)
            else:
                t = pool.tile([H, NP, WP], dt, name=f"ina_{key}", tag=f"ina_{key}")
                dma(t[:], xin[src_k, src_c_or_ranges, dy : dy + H])
                _touch(t[:, 0], 0, scalar_too=True)
            in_a[key] = t
            if two_byte:
                tb = pool.tile(list(t.shape), dt, name=f"inb_{key}", tag=f"inb_{key}")
                nc.scalar.copy(tb[:, :, 0 : WP - 1], t[:, :, 1:WP])
                _touch(tb[:, 0])
                in_b[key] = tb

        for k in range(3):
            for dy in range(KH):
                if pack == "cp128":
                    for m in range(NCHUNK):
                        _load((k, dy, m), k, _chunk_ranges(m), dy)
                else:
                    for c in range(C):
                        _load((k, dy, c), k, c, dy)

        kkt = pool.tile([NPART, 2 * nkk if pack == "cp128" else nkk], mybir.dt.float32, name="kkt", tag="kkt")
        dma(kkt[:], kkin[:])
        _touch(kkt, 0, scalar_too=True)

        accs = {}
        if pack == "cp128":
            # One tile per filter with the chunk index as a free dim, so the
            # output needs only NF=4 DMAs (<=8 HWDGE queues, no FIFO reuse wait).
            accf = {}
            for f in range(NF):
                accf[f] = pool.tile([128, NCHUNK, NP, W], dt, name=f"acc_{f}", tag=f"acc_{f}")
                for m in range(NCHUNK):
                    accs[f, m] = accf[f][:, m]
        else:
            for c in range(C):
                for f in range(NF):
                    accs[c, f] = pool.tile([H, NP, W], dt, name=f"acc_{c}_{f}", tag=f"acc_{c}_{f}")

        taps = [(k, dy, dx) for k in range(3) for dy in range(KH) for dx in range(KW)]

        # Unfused two-op path: tensor_scalar has a 4x fp16 uop and plain
        # tensor_tensor min a 2x one, while the fused scalar_tensor_tensor
        # only runs 1x — two instructions are cheaper than one.  gpsimd (no
        # scalar_tensor_tensor support) uses the same two-op shape.
        split_ops = two_byte and pack == "cp128" and CFG_SPLIT
        tmp_v = [pool.tile([128, NP, W], dt, name=f"tmpv_{i}", tag=f"tmpv_{i}") for i in range(4)] if split_ops else []
        tmp_g = [pool.tile([128, NP, W], dt, name=f"tmpg_{i}", tag=f"tmpg_{i}") for i in range(4)] if (split_ops and gpsimd_n > 0) else []

        def emit(ti, win_sel, acc, col):
            kk_ap = kkt[:, ti * ncols + col : ti * ncols + col + 1]
            on_gp = col >= ncols - gpsimd_n
            eng = nc.gpsimd if on_gp else nc.vector
            if ti == 0:
                eng.tensor_scalar(acc[:], win_sel, kk_ap, None, mybir.AluOpType.subtract)
            elif split_ops:
                tmp = (tmp_g if on_gp else tmp_v)[col % 4]
                eng.tensor_scalar(tmp[:], win_sel, kk_ap, None, mybir.AluOpType.subtract)
                eng.tensor_tensor(acc[:], tmp[:], acc[:], mybir.AluOpType.min)
            else:
                eng.scalar_tensor_tensor(
                    acc[:], win_sel, kk_ap, acc[:],
                    mybir.AluOpType.subtract, mybir.AluOpType.min,
                )

        for _rep in range(repeat):
          for ti, (k, dy, dx) in enumerate(taps):
            use_b = two_byte and (dx % 2 == 1)
            dxa = dx - 1 if use_b else dx
            if pack == "cp128":
                for f in range(NF):
                    for m in range(NCHUNK):
                        src = in_b[k, dy, m] if use_b else in_a[k, dy, m]
                        emit(ti, src[:, :, dxa : dxa + W], accs[f, m], f * NCHUNK + m)
            else:
                for c in range(C):
                    src = in_b[k, dy, c] if use_b else in_a[k, dy, c]
                    win = src[:, :, dxa : dxa + W]
                    for f in range(NF):
                        emit(ti, win, accs[c, f], c * NF + f)

        if pack == "cp128":
            # Channel sum happens on the host; just store the 12 acc tiles.
            for f in range(NF):
                # A Pool-engine touch absorbs the DVE dependency (1 wait), so
                # the SWDGE out-DMA dispatched next on the same sequencer needs
                # only its queue-FIFO wait.
                i = tctr[2] = tctr[2] + 1
                nc.gpsimd.tensor_scalar_add(touch_g[0:1, i : i + 1], accf[f][0:1, 0, 0, 0:1], 0.0)
                dma(yout[f], accf[f][:])

        else:
            out_t = pool.tile([H, NP, W, NF], mybir.dt.float32, name="out_t", tag="out_t")
            for f in range(NF):
                s1 = pool.tile([H, NP, W], mybir.dt.float32, name=f"s1_{f}", tag="s1", bufs=2)
                s2 = pool.tile([H, NP, W], mybir.dt.float32, name=f"s2_{f}", tag="s2", bufs=2)
                nc.vector.tensor_add(s1[:], accs[0, f][:], accs[1, f][:])
                nc.vector.tensor_add(s2[:], accs[2, f][:], accs[3, f][:])
                nc.vector.tensor_add(out_t[:, :, :, f], s1[:], s2[:])
            nc.sync.dma_start(yout[:], out_t[:])

    return nc


def _get_program(dtype_name, pack, gpsimd_n, repeat=1):
    key = (dtype_name, pack, gpsimd_n, repeat, CFG_SPLIT)
    if key not in _prog_cache:
        _prog_cache[key] = _build_program(dtype_name, pack, gpsimd_n, repeat)
    return _prog_cache[key]


def _krev(kernel):
    """[g, dy, dx, k, c, f] rotated/reversed SE, pure re-indexing of `kernel`."""
    k_ero = np.stack(
        [
            np.rot90(kernel[:, :, 2], k=3, axes=(0, 1)),
            kernel[:, :, 1],
            np.rot90(kernel[:, :, 0], k=1, axes=(0, 1)),
        ],
        axis=2,
    )
    krot = np.stack([np.rot90(k_ero, k=j, axes=(0, 1)) for j in range(4)], axis=0)
    return krot[:, ::-1, ::-1]


def _core_units(core):
    g = core // 2
    fh = core % 2
    return g, list(range(B)), list(range(fh * NF, fh * NF + NF))


def _make_in_map(x, kr, pack, core, np_dt, big, two_byte):
    g, bs, fs = _core_units(core)
    planes = np.full((3, C, HP, NP, WP), big, np.float32)
    for pi, b in enumerate(bs):
        for k in range(3):
            src = x[b, (g + k - 1) % 4]  # [H, W, C]
            planes[k, :, PAD : PAD + H, pi, PAD : PAD + W] = src.transpose(2, 0, 1)
    sel = kr[g][:, :, :, :, fs]  # [dy, dx, k, c, NF]
    taps_kcf = np.ascontiguousarray(sel.transpose(2, 0, 1, 3, 4))  # [k,dy,dx,c,NF]
    if pack == "cp128":
        # kk[p, (tap, f, m)] = kr[g, tap, c(m,p), f]
        tap_cf = taps_kcf.reshape(NTAP, C, NF)
        kk = np.empty((128, NTAP * NF * NCHUNK), np.float32)
        for m in range(NCHUNK):
            for (c, h0, h1, p0, p1) in _chunk_ranges(m):
                for ti in range(NTAP):
                    for f in range(NF):
                        kk[p0:p1, (ti * NF + f) * NCHUNK + m] = tap_cf[ti, c, f]
        kk = np.concatenate([kk, -kk], axis=1)
    else:
        kkflat = taps_kcf.reshape(-1)
        kk = np.ascontiguousarray(np.broadcast_to(kkflat, (H, kkflat.size)))
    return {"xin": planes.astype(np_dt), "kk": np.ascontiguousarray(kk)}


def _assemble(results, pack):
    out = np.zeros((B, G, H, W, F), np.float32)
    for core in range(N_CORES):
        g, bs, fs = _core_units(core)
        y = np.asarray(results[core]["yout"]).astype(np.float32)
        if pack == "cp128":
            # y: [NF, NCHUNK, 128, NP, W]; sum the c pieces into out
            for fi, f in enumerate(fs):
                for m in range(NCHUNK):
                    for (c, h0, h1, p0, p1) in _chunk_ranges(m):
                        for pi, b in enumerate(bs):
                            out[b, g, h0:h1, :, f] += y[fi, p0:p1, m, pi, :]
        else:
            for pi, b in enumerate(bs):
                out[b, g, :, :, fs[0] : fs[0] + len(fs)] = y[:, pi]
    return out


def kernel(x, kernel):
    x = np.ascontiguousarray(np.asarray(x, dtype=np.float32))
    se = np.ascontiguousarray(np.asarray(kernel, dtype=np.float32))
    dtype_name, pack, gpsimd_n = CFG_DTYPE, CFG_PACK, CFG_GPSIMD
    np_dt = _np_dtype(dtype_name)
    big = _DT[dtype_name][2]
    two_byte = dtype_name in ("fp16", "bf16")

    kr = _krev(se)  # [g, dy, dx, k, c, f]
    in_maps = [
        _make_in_map(x, kr, pack, core, np_dt, big, two_byte) for core in range(N_CORES)
    ]

    nc = _get_program(dtype_name, pack, gpsimd_n, CFG_REPEAT)
    res = run_bass_kernel_spmd(nc, in_maps, list(range(N_CORES)), trace=False)
    global LAST_RESULTS
    LAST_RESULTS = res
    return _assemble(res.results, pack)


# revision 32
# speedup vs baseline: 356.2520x; 1.4445x over previous
"""Trainium2 Bass kernel for ErosionP4 (P4 group-equivariant grayscale erosion).

Reference computation (shapes hardcoded):
  x: [B=4, G=4, H=96, W=96, C=4] fp32, kernel: [5, 5, 3, C=4, F=8] fp32
  out[b,g,h,w,f] = sum_c min_{k,dy,dx} ( ygp[b,g,k,h+dy,w+dx,c] - krev[g,dy,dx,k,c,f] )
  where ygp[b,g,k] = x[b, (g+k-1) mod 4] spatially padded with +inf and
  krev = the 4 planar rotations of the depth-rotated SE, spatially reversed.

Sharding: core -> (g = core//2, f-half = core%2).  Each core computes all 4
batches for one group-rotation g and 4 of the 8 filters.  All four batches
share the SE values for the core's g.

Packing "cp128": the (c, h) axes are flattened into a 384-row stream split
into 3 chunks of 128 partitions, so every DVE instruction runs with all 128
lanes busy.  The per-(tap,c,f) SE value varies across partitions within a
chunk, carried by the per-partition scalar operand.  The channel sum then
happens on the host (the c pieces are partition-misaligned on device).

Per (tap, f, chunk) the erosion update acc = min(window - kk, acc) runs as
two DVE ops — tensor_scalar subtract (4x fp16 uop) + tensor_tensor min (2x
fp16 uop), HW-measured at 373 us vs 470 us for the fused 1x
scalar_tensor_tensor (CFG_SPLIT=0 fallback).
"""

import os
from contextlib import ExitStack

import numpy as np

import concourse.bass as bass
import concourse.mybir as mybir
import concourse.tile as tile
from concourse.bass_utils import run_bass_kernel_spmd

B, G, H, W, C = 4, 4, 96, 96, 4
KH, KW, F = 5, 5, 8
PAD = 2
HP, WP = H + PAD * 2, W + PAD * 2  # 100, 100
NTAP = 3 * KH * KW  # 75
N_CORES = 8
NP = 4  # batches per core
NF = F // 2  # filters per core
NCHUNK = 3  # ceil(C*H / 128)

# Configuration (module-level so experiments can flip them; defaults = best).
CFG_DTYPE = os.environ.get("KCFG_DTYPE", "fp16")  # fp32 | fp16 | bf16
CFG_PACK = os.environ.get("KCFG_PACK", "cp128")  # h96 | cp128
CFG_GPSIMD = int(os.environ.get("KCFG_GPSIMD", "0"))  # of NF*NCHUNK (cp128) or C*NF (h96) columns on gpsimd
CFG_REPEAT = int(os.environ.get("KCFG_REPEAT", "1"))  # repeat compute on-device (timing slope runs)
CFG_SPLIT = int(os.environ.get("KCFG_SPLIT", "1"))  # 1: unfused ts+tt (2x/4x uops); 0: fused scalar_tensor_tensor
CFG_ACTSUB = int(os.environ.get("KCFG_ACTSUB", "0"))  # cols whose subtract runs on the Scalar engine

_DT = {
    "fp32": (mybir.dt.float32, np.float32, 1e30),
    "fp16": (mybir.dt.float16, np.float16, 30000.0),
    "bf16": (mybir.dt.bfloat16, None, 1e30),
}

_prog_cache = {}
LAST_RESULTS = None


def _np_dtype(name):
    if name == "bf16":
        import ml_dtypes

        return np.dtype(ml_dtypes.bfloat16)
    return np.dtype(_DT[name][1])


def _chunk_ranges(m):
    """(c, h0, h1, p0, p1) pieces of stream rows [128m, 128(m+1))."""
    out = []
    r = 128 * m
    while r < 128 * (m + 1):
        c, h = r // H, r % H
        h1 = min(H, h + 128 * (m + 1) - r)
        out.append((c, h, h1, r - 128 * m, r - 128 * m + (h1 - h)))
        r += h1 - h
    return out


def _build_program(dtype_name, pack, gpsimd_n, repeat=1):
    dt, _, _ = _DT[dtype_name]
    two_byte = dtype_name in ("fp16", "bf16")
    # The kernel-tail Drain must wait on every sem lane used; with 8 SWDGE
    # lanes + 3 engines it exceeds the CTRL struct's sync-wait capacity.
    # Cap the SWDGE completion-sem lanes for this build.
    import concourse.tile_sem_assignment as _tsa

    _orig_swdge = _tsa.NUM_SWDGE_GLOBAL_SEMS
    _tsa.NUM_SWDGE_GLOBAL_SEMS = 4
    try:
        return _build_program_inner(dtype_name, pack, gpsimd_n, dt, two_byte, repeat)
    finally:
        _tsa.NUM_SWDGE_GLOBAL_SEMS = _orig_swdge


class _SplitDrainTC(tile.TileContext):
    """TileContext whose kernel-tail drain is split into one drain per sem
    lane: the stock single Drain carries a wait for every lane used, which
    overflows the CTRL struct's sync-wait encoding on this compiler."""

    def _drain_and_barrier(self, tick_clock, wait_clock):
        from concourse.tile_sem_assignment import N_PROCS
        from concourse.vector_clock import ScopedClock, VectorClock

        gc = tick_clock.global_clock
        ticks = [gc[p] for p in range(N_PROCS)]
        for p in range(N_PROCS):
            if ticks[p] <= 0:
                continue
            sub = [ticks[q] if q == p else 0 for q in range(N_PROCS)]
            d = self.nc.sync.drain()
            wait_clock.add_sem_waits(d.ins, ScopedClock({None: VectorClock(sub)}))

        self.nc.all_engine_barrier()
        assert self.sems is not None
        popped = self.nc._tile_sem_poison_stack.pop()
        assert popped is self._sem_poison
        self.nc.clear_and_free_semaphores(list(self.sems.allocated().values()))
        self.nc.all_engine_barrier()


def _build_program_inner(dtype_name, pack, gpsimd_n, dt, two_byte, repeat=1):
    nc = bass.Bass()
    # Input planes: [k, c, h_pad, pair, w_pad]; for 2-byte dtypes a second
    # copy shifted by one w element keeps odd-dx windows 4B-aligned (DVE
    # 2x packed mode needs aligned step-1 operands).
    xin = nc.declare_dram_parameter("xin", [3, C, HP, NP, WP], dt, isOutput=False)

    if pack == "cp128":
        ncols = NF * NCHUNK  # engine-split granularity per tap
        nkk = NTAP * NF * NCHUNK
        kkin = nc.declare_dram_parameter("kk", [128, 2 * nkk], mybir.dt.float32, isOutput=False)
        yout = nc.declare_dram_parameter("yout", [NF, 128, NCHUNK, NP, W], dt, isOutput=True)
    else:
        ncols = C * NF
        nkk = NTAP * ncols
        kkin = nc.declare_dram_parameter("kk", [H, nkk], mybir.dt.float32, isOutput=False)
        yout = nc.declare_dram_parameter("yout", [H, NP, W, NF], mybir.dt.float32, isOutput=True)

    with _SplitDrainTC(nc) as tc, ExitStack() as ctx:
        pool = ctx.enter_context(tc.tile_pool(name="main", bufs=1))

        # Compute-instruction ISA slots can encode only ONE sync wait, so
        # "touch" every DMA'd region with a trivial op on each consuming
        # engine right after its DMA (one wait each); later compute
        # instructions then inherit the dependency through engine program
        # order and carry no waits of their own.
        # Distinct destination slots per touch: a shared destination would be a
        # same-engine WAW hazard, which costs this instruction's single wait slot.
        touch_v = pool.tile([1, 512], mybir.dt.float32, name="touch_v", tag="touch_v")
        touch_s = pool.tile([1, 512], mybir.dt.float32, name="touch_s", tag="touch_s")
        touch_g = pool.tile([1, 512], mybir.dt.float32, name="touch_g", tag="touch_g")
        tctr = [0, 0, 0]

        def _touch(t, p0=0, scalar_too=False):
            src = t[p0 : p0 + 1, 0:1]
            i = tctr[0] = tctr[0] + 1
            nc.vector.tensor_scalar_add(touch_v[0:1, i : i + 1], src, 0.0)
            if scalar_too and two_byte:
                i = tctr[1] = tctr[1] + 1
                nc.scalar.copy(touch_s[0:1, i : i + 1], src)
            i = tctr[2] = tctr[2] + 1
            nc.gpsimd.tensor_scalar_add(touch_g[0:1, i : i + 1], src, 0.0)

        NPART = 128 if pack == "cp128" else H

        # One HWDGE dma_start fans out over several HW queues, so a consumer
        # would need more sync waits than compute-instruction ISA slots can
        # encode; the software DGE (gpsimd engine) uses a single queue.
        dma = nc.gpsimd.dma_start

        # Compute-engine SBUF reads must start at partition 0/32/64/96, so the
        # dy window shift cannot be a partition offset — keep one dy-shifted
        # copy per (k, dy, chunk) (cp128) / (k, dy, c) (h96), sliced from HBM.
        # The odd-dx alignment copy (in_b = in_a shifted one element left) is
        # built by the otherwise-idle Scalar engine instead of more DMAs.
        in_a = {}
        in_b = {}

        def _load(key, src_k, src_c_or_ranges, dy):
            if pack == "cp128":
                t = pool.tile([128, NP, WP], dt, name=f"ina_{key}", tag=f"ina_{key}")
                for (c, h0, h1, p0, p1) in src_c_or_ranges:
                    dma(t[p0:p1], xin[src_k, c, h0 + dy : h1 + dy])
                    _touch(t[:, 0], p0, scalar_too=True)
            else:
                t = pool.tile([H, NP, WP], dt, name=f"ina_{key}", tag=f"ina_{key}")
                dma(t[:], xin[src_k, src_c_or_ranges, dy : dy + H])
                _touch(t[:, 0], 0, scalar_too=True)
            in_a[key] = t
            if two_byte:
                tb = pool.tile(list(t.shape), dt, name=f"inb_{key}", tag=f"inb_{key}")
                nc.scalar.copy(tb[:, :, 0 : WP - 1], t[:, :, 1:WP])
                _touch(tb[:, 0])
                in_b[key] = tb

        for k in range(3):
            for dy in range(KH):
                if pack == "cp128":
                    for m in range(NCHUNK):
                        _load((k, dy, m), k, _chunk_ranges(m), dy)
                else:
                    for c in range(C):
                        _load((k, dy, c), k, c, dy)

        kkt = pool.tile([NPART, 2 * nkk if pack == "cp128" else nkk], mybir.dt.float32, name="kkt", tag="kkt")
        dma(kkt[:], kkin[:])
        _touch(kkt, 0, scalar_too=True)

        accs = {}
        if pack == "cp128":
            # One tile per filter with the chunk index as a free dim, so the
            # output needs only NF=4 DMAs (<=8 HWDGE queues, no FIFO reuse wait).
            accf = {}
            for f in range(NF):
                accf[f] = pool.tile([128, NCHUNK, NP, W], dt, name=f"acc_{f}", tag=f"acc_{f}")
                for m in range(NCHUNK):
                    accs[f, m] = accf[f][:, m]
        else:
            for c in range(C):
                for f in range(NF):
                    accs[c, f] = pool.tile([H, NP, W], dt, name=f"acc_{c}_{f}", tag=f"acc_{c}_{f}")

        taps = [(k, dy, dx) for k in range(3) for dy in range(KH) for dx in range(KW)]

        # Unfused two-op path: tensor_scalar has a 4x fp16 uop and plain
        # tensor_tensor min a 2x one, while the fused scalar_tensor_tensor
        # only runs 1x — two instructions are cheaper than one.  gpsimd (no
        # scalar_tensor_tensor support) uses the same two-op shape.
        split_ops = two_byte and pack == "cp128" and CFG_SPLIT
        n_act = CFG_ACTSUB if split_ops else 0
        tmp_v = [pool.tile([128, NP, W], dt, name=f"tmpv_{i}", tag=f"tmpv_{i}") for i in range(4)] if split_ops else []
        tmp_g = [pool.tile([128, NP, W], dt, name=f"tmpg_{i}", tag=f"tmpg_{i}") for i in range(4)] if (split_ops and gpsimd_n > 0) else []
        tmp_a = [pool.tile([128, NP, W], dt, name=f"tmpa_{i}", tag=f"tmpa_{i}") for i in range(16)] if n_act else []
        actr = [0]

        def emit(ti, win_sel, acc, col):
            kk_ap = kkt[:, ti * ncols + col : ti * ncols + col + 1]
            on_gp = col >= ncols - gpsimd_n
            on_act = (not on_gp) and n_act > 0 and col >= ncols - gpsimd_n - n_act
            eng = nc.gpsimd if on_gp else nc.vector
            if ti == 0:
                eng.tensor_scalar(acc[:], win_sel, kk_ap, None, mybir.AluOpType.subtract)
            elif split_ops:
                if on_act:
                    # ACT computes win - kk via its per-partition bias (the
                    # negated kk in the second kk half); DVE keeps only the
                    # 2x tensor_tensor min.
                    negkk_ap = kkt[:, nkk + ti * ncols + col : nkk + ti * ncols + col + 1]
                    tmp = tmp_a[actr[0] % 16]
                    actr[0] += 1
                    nc.scalar.activation(
                        tmp[:], win_sel, mybir.ActivationFunctionType.Identity, bias=negkk_ap
                    )
                    nc.vector.tensor_tensor(acc[:], tmp[:], acc[:], mybir.AluOpType.min)
                else:
                    tmp = (tmp_g if on_gp else tmp_v)[col % 4]
                    eng.tensor_scalar(tmp[:], win_sel, kk_ap, None, mybir.AluOpType.subtract)
                    eng.tensor_tensor(acc[:], tmp[:], acc[:], mybir.AluOpType.min)
            else:
                eng.scalar_tensor_tensor(
                    acc[:], win_sel, kk_ap, acc[:],
                    mybir.AluOpType.subtract, mybir.AluOpType.min,
                )

        for _rep in range(repeat):
          for ti, (k, dy, dx) in enumerate(taps):
            use_b = two_byte and (dx % 2 == 1)
            dxa = dx - 1 if use_b else dx
            if pack == "cp128":
                if n_act and ti > 0:
                    # ACT absorber: observe DVE's latest acc tick so ACT's ring
                    # rewrites carry only their same-engine WAW wait.
                    i = tctr[1] = tctr[1] + 1
                    nc.scalar.copy(touch_s[0:1, i : i + 1], accs[NF - 1, NCHUNK - 1][0:1, 0, 0:1])
                    # emit this tap's ACT subs first, then a DVE absorber on the
                    # last one so the tt-mins carry only their acc-chain wait.
                    for f in range(NF):
                        for m in range(NCHUNK):
                            col = f * NCHUNK + m
                            if col >= ncols - gpsimd_n - n_act and col < ncols - gpsimd_n:
                                negkk_ap = kkt[:, nkk + ti * ncols + col : nkk + ti * ncols + col + 1]
                                tmp = tmp_a[actr[0] % 16]
                                actr[0] += 1
                                srct = in_b[k, dy, m] if use_b else in_a[k, dy, m]
                                nc.scalar.activation(
                                    tmp[:], srct[:, :, dxa : dxa + W],
                                    mybir.ActivationFunctionType.Identity, bias=negkk_ap,
                                )
                    i = tctr[0] = tctr[0] + 1
                    nc.vector.tensor_scalar_add(
                        touch_v[0:1, i : i + 1], tmp_a[(actr[0] - 1) % 16][0:1, 0, 0:1], 0.0
                    )
                    for f in range(NF):
                        for m in range(NCHUNK):
                            col = f * NCHUNK + m
                            if col >= ncols - gpsimd_n - n_act and col < ncols - gpsimd_n:
                                tmp = tmp_a[(actr[0] - (ncols - gpsimd_n - (ncols - gpsimd_n - n_act)) + (col - (ncols - gpsimd_n - n_act))) % 16]
                                nc.vector.tensor_tensor(accs[f, m][:], tmp[:], accs[f, m][:], mybir.AluOpType.min)
                            elif col < ncols - gpsimd_n - n_act or col >= ncols - gpsimd_n:
                                srct = in_b[k, dy, m] if use_b else in_a[k, dy, m]
                                emit(ti, srct[:, :, dxa : dxa + W], accs[f, m], col)
                else:
                    for f in range(NF):
                        for m in range(NCHUNK):
                            src = in_b[k, dy, m] if use_b else in_a[k, dy, m]
                            emit(ti, src[:, :, dxa : dxa + W], accs[f, m], f * NCHUNK + m)
            else:
                for c in range(C):
                    src = in_b[k, dy, c] if use_b else in_a[k, dy, c]
                    win = src[:, :, dxa : dxa + W]
                    for f in range(NF):
                        emit(ti, win, accs[c, f], c * NF + f)

        if pack == "cp128":
            # Channel sum happens on the host; just store the 12 acc tiles.
            for f in range(NF):
                # A Pool-engine touch absorbs the DVE dependency (1 wait), so
                # the SWDGE out-DMA dispatched next on the same sequencer needs
                # only its queue-FIFO wait.
                i = tctr[2] = tctr[2] + 1
                nc.gpsimd.tensor_scalar_add(touch_g[0:1, i : i + 1], accf[f][0:1, 0, 0, 0:1], 0.0)
                dma(yout[f], accf[f][:])

        else:
            out_t = pool.tile([H, NP, W, NF], mybir.dt.float32, name="out_t", tag="out_t")
            for f in range(NF):
                s1 = pool.tile([H, NP, W], mybir.dt.float32, name=f"s1_{f}", tag="s1", bufs=2)
                s2 = pool.tile([H, NP, W], mybir.dt.float32, name=f"s2_{f}", tag="s2", bufs=2)
                nc.vector.tensor_add(s1[:], accs[0, f][:], accs[1, f][:])
                nc.vector.tensor_add(s2[:], accs[2, f][:], accs[3, f][:])
                nc.vector.tensor_add(out_t[:, :, :, f], s1[:], s2[:])
            nc.sync.dma_start(yout[:], out_t[:])

    return nc


def _get_program(dtype_name, pack, gpsimd_n, repeat=1):
    key = (dtype_name, pack, gpsimd_n, repeat, CFG_SPLIT, CFG_ACTSUB)
    if key not in _prog_cache:
        _prog_cache[key] = _build_program(dtype_name, pack, gpsimd_n, repeat)
    return _prog_cache[key]


def _krev(kernel):
    """[g, dy, dx, k, c, f] rotated/reversed SE, pure re-indexing of `kernel`."""
    k_ero = np.stack(
        [
            np.rot90(kernel[:, :, 2], k=3, axes=(0, 1)),
            kernel[:, :, 1],
            np.rot90(kernel[:, :, 0], k=1, axes=(0, 1)),
        ],
        axis=2,
    )
    krot = np.stack([np.rot90(k_ero, k=j, axes=(0, 1)) for j in range(4)], axis=0)
    return krot[:, ::-1, ::-1]


def _core_units(core):
    g = core // 2
    fh = core % 2
    return g, list(range(B)), list(range(fh * NF, fh * NF + NF))


def _make_in_map(x, kr, pack, core, np_dt, big, two_byte):
    g, bs, fs = _core_units(core)
    planes = np.full((3, C, HP, NP, WP), big, np.float32)
    for pi, b in enumerate(bs):
        for k in range(3):
            src = x[b, (g + k - 1) % 4]  # [H, W, C]
            planes[k, :, PAD : PAD + H, pi, PAD : PAD + W] = src.transpose(2, 0, 1)
    sel = kr[g][:, :, :, :, fs]  # [dy, dx, k, c, NF]
    taps_kcf = np.ascontiguousarray(sel.transpose(2, 0, 1, 3, 4))  # [k,dy,dx,c,NF]
    if pack == "cp128":
        # kk[p, (tap, f, m)] = kr[g, tap, c(m,p), f]
        tap_cf = taps_kcf.reshape(NTAP, C, NF)
        kk = np.empty((128, NTAP * NF * NCHUNK), np.float32)
        for m in range(NCHUNK):
            for (c, h0, h1, p0, p1) in _chunk_ranges(m):
                for ti in range(NTAP):
                    for f in range(NF):
                        kk[p0:p1, (ti * NF + f) * NCHUNK + m] = tap_cf[ti, c, f]
        kk = np.concatenate([kk, -kk], axis=1)
    else:
        kkflat = taps_kcf.reshape(-1)
        kk = np.ascontiguousarray(np.broadcast_to(kkflat, (H, kkflat.size)))
    return {"xin": planes.astype(np_dt), "kk": np.ascontiguousarray(kk)}


def _assemble(results, pack):
    out = np.zeros((B, G, H, W, F), np.float32)
    for core in range(N_CORES):
        g, bs, fs = _core_units(core)
        y = np.asarray(results[core]["yout"]).astype(np.float32)
        if pack == "cp128":
            # y: [NF, NCHUNK, 128, NP, W]; sum the c pieces into out
            for fi, f in enumerate(fs):
                for m in range(NCHUNK):
                    for (c, h0, h1, p0, p1) in _chunk_ranges(m):
                        for pi, b in enumerate(bs):
                            out[b, g, h0:h1, :, f] += y[fi, p0:p1, m, pi, :]
        else:
            for pi, b in enumerate(bs):
                out[b, g, :, :, fs[0] : fs[0] + len(fs)] = y[:, pi]
    return out


def kernel(x, kernel):
    x = np.ascontiguousarray(np.asarray(x, dtype=np.float32))
    se = np.ascontiguousarray(np.asarray(kernel, dtype=np.float32))
    dtype_name, pack, gpsimd_n = CFG_DTYPE, CFG_PACK, CFG_GPSIMD
    np_dt = _np_dtype(dtype_name)
    big = _DT[dtype_name][2]
    two_byte = dtype_name in ("fp16", "bf16")

    kr = _krev(se)  # [g, dy, dx, k, c, f]
    in_maps = [
        _make_in_map(x, kr, pack, core, np_dt, big, two_byte) for core in range(N_CORES)
    ]

    nc = _get_program(dtype_name, pack, gpsimd_n, CFG_REPEAT)
    res = run_bass_kernel_spmd(nc, in_maps, list(range(N_CORES)), trace=False)
    global LAST_RESULTS
    LAST_RESULTS = res
    return _assemble(res.results, pack)
